# revision 8
# baseline (speedup 1.0000x reference)
"""Bucket-random causal attention kernel for Trainium2 (8 NeuronCores, SPMD).

Strategy: data-parallel over batch B=8 (one batch element per core).
Host-side prep per core b:
  - qT  = q[:, b, :].T           [D, L]  f16   (pre-transposed so the contraction dim lands on partitions)
  - ksT = k[sampled_index, b].T  [D, S]  f16   (bucket gather done host-side: pure data movement)
  - vsT = v[sampled_index, b].T  [D, S]  f16
  - mask[i, s] = 1 if (sampled_index[s] <= i and s < mkl_b) else 0   [L, S] f16
  - weights pre-transposed (torch linear computes x @ W.T, we feed W.T directly),
    with the 1/sqrt(dh) score scale folded into Wq/bq.
Device kernel per core (all matmul operands fp16, fp32 PSUM accumulation):
  kp/vp projection once; then stream 16 blocks of 256 query rows:
  qp -> scores -> masked softmax (exp on ACT, mask/Z/recip/norm on DVE)
  -> DMA-transpose of attn -> ctx -> out-projection (bias via K=1 matmul row) -> fp32 out.
The program is specialized on sampled_index: query block blk only attends to the
first s_sz[blk] sampled keys (causal prefix), shrinking scores/softmax/ctx work.
"""

import numpy as np

L = 4096
B = 8
D = 1024
H = 16
DH = 64
S = 128
NBLK = 16
IB = 256  # query rows per block (2 subblocks of 128)

_CACHE = {}


def _s_sizes(sampled):
    """Static per-block prefix length of sampled keys (rounded up to 32)."""
    sampled = np.asarray(sampled)
    if np.any(np.diff(sampled) < 0):
        return [S] * NBLK  # unsorted: no prefix structure, use all keys
    out = []
    for blk in range(NBLK):
        i_hi = blk * IB + IB - 1
        count = int((sampled <= i_hi).sum())
        out.append(min(S, max(32, ((count + 31) // 32) * 32)))
    return out


def _build(s_sz, stages=9):
    import concourse.bacc as bacc
    import concourse.tile as tile
    import concourse.mybir as mybir

    f32 = mybir.dt.float32
    f16 = mybir.dt.float16
    Exp = mybir.ActivationFunctionType.Exp
    AX = mybir.AxisListType.X
    ADD = mybir.AluOpType.add

    nc = bacc.Bacc("TRN2", target_bir_lowering=False, debug=False, num_devices=8)

    qT_d = nc.dram_tensor("qT", [D, L], f16, kind="ExternalInput")
    ksT_d = nc.dram_tensor("ksT", [D, S], f16, kind="ExternalInput")
    vsT_d = nc.dram_tensor("vsT", [D, S], f16, kind="ExternalInput")
    wq_d = nc.dram_tensor("wq", [D, D], f16, kind="ExternalInput")   # Wq.T / 8
    wk_d = nc.dram_tensor("wk", [D, D], f16, kind="ExternalInput")   # Wk.T
    wv_d = nc.dram_tensor("wv", [D, D], f16, kind="ExternalInput")   # Wv.T
    wo_d = nc.dram_tensor("wo", [D, D], f16, kind="ExternalInput")   # out_w.T
    bq_d = nc.dram_tensor("bq", [128, 8], f32, kind="ExternalInput")  # col c = bq[c*128:(c+1)*128]/8
    bk_d = nc.dram_tensor("bk", [1, D], f16, kind="ExternalInput")
    bv_d = nc.dram_tensor("bv", [1, D], f16, kind="ExternalInput")
    ob_d = nc.dram_tensor("ob", [1, D], f16, kind="ExternalInput")
    mask_d = nc.dram_tensor("mask", [L, S], f16, kind="ExternalInput")
    out_d = nc.dram_tensor("out", [L, D], f32, kind="ExternalOutput")

    with tile.TileContext(nc) as tc:
        with (
            tc.tile_pool(name="const", bufs=1) as cpool,
            tc.tile_pool(name="qt", bufs=2) as qt_pool,
            tc.tile_pool(name="qp", bufs=2) as qp_pool,
            tc.tile_pool(name="exp", bufs=2) as exp_pool,
            tc.tile_pool(name="att", bufs=2) as att_pool,
            tc.tile_pool(name="zr", bufs=2) as zr_pool,
            tc.tile_pool(name="cxs", bufs=2) as cxs_pool,
            tc.tile_pool(name="osb", bufs=2) as osb_pool,
            tc.tile_pool(name="msk", bufs=2) as msk_pool,
            tc.tile_pool(name="ps_qp", bufs=2, space="PSUM") as ps_qp,
            tc.tile_pool(name="ps_sc", bufs=2, space="PSUM") as ps_sc,
            tc.tile_pool(name="ps_cx", bufs=2, space="PSUM") as ps_cx,
            tc.tile_pool(name="ps_out", bufs=2, space="PSUM") as ps_out,
        ):
            # ---------- constants ----------
            wq_sb = cpool.tile([128, 8, D], f16)
            wo_sb = cpool.tile([128, 8, D], f16)
            for c in range(8):
                nc.sync.dma_start(wq_sb[:, c, :], wq_d.ap()[c * 128:(c + 1) * 128, :])
                nc.sync.dma_start(wo_sb[:, c, :], wo_d.ap()[c * 128:(c + 1) * 128, :])
            ksT_sb = cpool.tile([128, 8, S], f16)
            vsT_sb = cpool.tile([128, 8, S], f16)
            for c in range(8):
                nc.sync.dma_start(ksT_sb[:, c, :], ksT_d.ap()[c * 128:(c + 1) * 128, :])
                nc.sync.dma_start(vsT_sb[:, c, :], vsT_d.ap()[c * 128:(c + 1) * 128, :])
            bq_sb = cpool.tile([128, 8], f32)
            nc.sync.dma_start(bq_sb[:], bq_d.ap()[:])
            bk_sb = cpool.tile([1, D], f16)
            bv_sb = cpool.tile([1, D], f16)
            ob_sb = cpool.tile([1, D], f16)
            nc.sync.dma_start(bk_sb[:], bk_d.ap()[:])
            nc.sync.dma_start(bv_sb[:], bv_d.ap()[:])
            nc.sync.dma_start(ob_sb[:], ob_d.ap()[:])
            ones_sb = cpool.tile([1, 128], f16)
            nc.vector.memset(ones_sb[:], 1.0)

            kpT_sb = cpool.tile([128, 8, S], f16)   # kp.T chunks: [d' in chunk, chunk, s]
            vp_sb = cpool.tile([128, D], f16)       # vp: [s, d']
            kp_sb = cpool.tile([128, D], f16)       # kp: [s, d'] (pre-transpose temp)
            # zero-padded variants so every matmul stays in full 128x128 PE mode
            # (operand partition offsets trigger PE row-tiling; concurrent row
            # tiles share a PSUM bank and crash the device)
            kpT_lo = cpool.tile([128, 8, S], f16)   # rows 64:128 zeroed (even heads)
            kpT_hi = cpool.tile([128, 8, S], f16)   # rows 0:64 zeroed (odd heads)
            vp_ev = cpool.tile([128, D], f16)       # odd-head columns zeroed
            vp_od = cpool.tile([128, D], f16)       # even-head columns zeroed

            # ---------- k/v projection (once) ----------
            for which, xT_sb, w_d, b_sb, dst in (("k", ksT_sb, wk_d, bk_sb, kp_sb),
                                                 ("v", vsT_sb, wv_d, bv_sb, vp_sb)):
                w_sb = cpool.tile([128, 8, D], f16, tag="wkv", name=f"w_{which}")
                for c in range(8):
                    nc.sync.dma_start(w_sb[:, c, :], w_d.ap()[c * 128:(c + 1) * 128, :])
                for half in range(2):
                    pp = ps_out.tile([128, 512], f32, name="pp", tag="po")
                    for c in range(8):
                        nc.tensor.matmul(pp[:], xT_sb[:, c, :], w_sb[:, c, half * 512:(half + 1) * 512],
                                         start=(c == 0), stop=False)
                    nc.tensor.matmul(pp[:], ones_sb[:, 0:S], b_sb[:, half * 512:(half + 1) * 512],
                                     start=False, stop=True)
                    nc.any.tensor_copy(dst[:, half * 512:(half + 1) * 512], pp[:])
            for c in range(8):  # kp -> kpT via DMA transpose
                nc.sync.dma_start_transpose(kpT_sb[:, c, :], kp_sb[:, c * 128:(c + 1) * 128])
            nc.vector.memset(kpT_lo[:], 0.0)
            nc.vector.memset(kpT_hi[:], 0.0)
            nc.vector.tensor_copy(kpT_lo[0:64, :, :], kpT_sb[0:64, :, :])
            nc.vector.tensor_copy(kpT_hi[64:128, :, :], kpT_sb[64:128, :, :])
            nc.vector.memset(vp_ev[:], 0.0)
            nc.vector.memset(vp_od[:], 0.0)
            vp3 = vp_sb.rearrange("p (h x) -> p h x", x=64)
            nc.vector.tensor_copy(vp_ev.rearrange("p (h x) -> p h x", x=64)[:, 0:16:2, :],
                                  vp3[:, 0:16:2, :])
            nc.vector.tensor_copy(vp_od.rearrange("p (h x) -> p h x", x=64)[:, 1:16:2, :],
                                  vp3[:, 1:16:2, :])

            # ---------- main loop over query blocks ----------
            for blk in range(NBLK):
                i0 = blk * IB
                ss = s_sz[blk]
                qt = qt_pool.tile([128, 8, IB], f16, name="qt", tag="qt")
                for c in range(8):
                    nc.sync.dma_start(qt[:, c, :], qT_d.ap()[c * 128:(c + 1) * 128, i0:i0 + IB])
                m4 = msk_pool.tile([128, 2, 4, S], f16, name="m4", tag="m4")
                for j in range(2):
                    for r in range(4):
                        nc.sync.dma_start(m4[:, j, r, 0:ss],
                                          mask_d.ap()[i0 + j * 128:i0 + (j + 1) * 128, 0:ss])

                # q projection: qp.T chunks [d' in chunk, chunk, i]
                if stages < 1:
                    continue
                qp_f = qp_pool.tile([128, 8, IB], f16, name="qp_f", tag="qp_f")
                for oc in range(8):
                    pq = ps_qp.tile([128, IB], f32, name="pq", tag="pq")
                    for kc in range(8):
                        nc.tensor.matmul(pq[:], wq_sb[:, kc, oc * 128:(oc + 1) * 128], qt[:, kc, :],
                                         start=(kc == 0), stop=(kc == 7))
                    nc.any.tensor_scalar_add(qp_f[:, oc, :], pq[:], bq_sb[:, oc:oc + 1])

                if stages < 2:
                    continue
                cx_tiles = []
                for j in range(2):
                    ex = exp_pool.tile([128, H, S], f16, name="ex", tag="ex")
                    zz = zr_pool.tile([128, H], f32, name="zz", tag="zz")
                    rr = zr_pool.tile([128, H], f32, name="rr", tag="rr")
                    if ss < S and stages >= 5:
                        # tail cols feed the (full-width) dma transpose; zero them
                        nc.gpsimd.memset(ex[:, :, ss:S], 0.0)
                    for g in range(4):
                        sc = ps_sc.tile([128, 4, S], f32, name="sc", tag="sc")
                        for gi in range(4):
                            h = g * 4 + gi
                            hp, hc = h % 2, h // 2
                            kside = kpT_lo if hp == 0 else kpT_hi
                            nc.tensor.matmul(sc[:, gi, 0:ss],
                                             qp_f[:, hc, j * 128:(j + 1) * 128],
                                             kside[:, hc, 0:ss],
                                             start=True, stop=True)
                        if stages < 3:
                            continue
                        nc.scalar.activation(ex[:, g * 4:(g + 1) * 4, 0:ss], sc[:, :, 0:ss], Exp)
                        if stages < 4:
                            continue
                        nc.vector.tensor_mul(ex[:, g * 4:(g + 1) * 4, 0:ss],
                                             ex[:, g * 4:(g + 1) * 4, 0:ss], m4[:, j, :, 0:ss])
                        nc.vector.tensor_reduce(zz[:, g * 4:(g + 1) * 4],
                                                ex[:, g * 4:(g + 1) * 4, 0:ss], AX, ADD)
                    if stages >= 4:
                        nc.vector.reciprocal(rr[:], zz[:])
                        for h in range(H):
                            nc.vector.tensor_scalar_mul(ex[:, h, 0:ss], ex[:, h, 0:ss], rr[:, h:h + 1])
                    # transpose attn -> [s, i] (full 128 cols: xbar needs %128; tail rows unread)
                    if stages < 5:
                        continue
                    at = att_pool.tile([128, H, S], f16, name="at", tag="at")
                    for h in range(H):
                        nc.sync.dma_start_transpose(at[:, h, :], ex[:, h, :])
                    # ctx: ctxT[d', i] quadrants [d' pair-chunk, (c_off, j, i)]
                    if stages < 6:
                        continue
                    for cp in range(4):
                        if j == 0:
                            cx_tiles.append(ps_cx.tile([128, 2, 2, 128], f32, name="cx", tag="cx"))
                        cx = cx_tiles[cp]
                        for co in range(2):
                            c = cp * 2 + co
                            nc.tensor.matmul(cx[:, co, j, :],
                                             vp_ev[:, c * 128:(c + 1) * 128], at[:, 2 * c, :],
                                             start=True, stop=False)
                            nc.tensor.matmul(cx[:, co, j, :],
                                             vp_od[:, c * 128:(c + 1) * 128], at[:, 2 * c + 1, :],
                                             start=False, stop=True)
                if stages < 7:
                    continue
                ctxT = cxs_pool.tile([128, 8, IB], f16, name="ctxT", tag="ctxT")
                for cp in range(4):
                    nc.any.tensor_copy(ctxT[:, cp * 2:cp * 2 + 2, :], cx_tiles[cp][:])

                # out projection
                if stages < 8:
                    continue
                for j in range(2):
                    for eh in range(2):
                        po = ps_out.tile([128, 512], f32, name="po", tag="po")
                        for c in range(8):
                            nc.tensor.matmul(po[:], ctxT[:, c, j * 128:(j + 1) * 128],
                                             wo_sb[:, c, eh * 512:(eh + 1) * 512],
                                             start=(c == 0), stop=False)
                        nc.tensor.matmul(po[:], ones_sb[:], ob_sb[:, eh * 512:(eh + 1) * 512],
                                         start=False, stop=True)
                        o_sb = osb_pool.tile([128, 512], f32, name="osb", tag="osb")
                        nc.any.tensor_copy(o_sb[:], po[:])
                        nc.sync.dma_start(
                            out_d.ap()[i0 + j * 128:i0 + (j + 1) * 128, eh * 512:(eh + 1) * 512],
                            o_sb[:])
    nc.compile()
    return nc


def _prep_inputs(q, k, v, key_length, sampled_index, in_proj_weight, in_proj_bias, out_w, out_b):
    f16 = np.float16
    sampled = np.asarray(sampled_index)
    klen = np.asarray(key_length)
    Wq = in_proj_weight[0:D]
    Wk = in_proj_weight[D:2 * D]
    Wv = in_proj_weight[2 * D:3 * D]
    bq = in_proj_bias[0:D]
    bk = in_proj_bias[D:2 * D]
    bv = in_proj_bias[2 * D:3 * D]
    scale = np.float32(1.0 / np.sqrt(DH))

    wq_t = np.ascontiguousarray((Wq.T * scale).astype(f16))
    wk_t = np.ascontiguousarray(Wk.T.astype(f16))
    wv_t = np.ascontiguousarray(Wv.T.astype(f16))
    wo_t = np.ascontiguousarray(out_w.T.astype(f16))
    bq8 = np.ascontiguousarray((bq * scale).astype(np.float32).reshape(8, 128).T)
    bk_r = bk.astype(f16).reshape(1, D)
    bv_r = bv.astype(f16).reshape(1, D)
    ob_r = out_b.astype(f16).reshape(1, D)

    mkl = (sampled[None, :] < klen[:, None]).sum(axis=1)  # [B]
    causal_ok = sampled[None, :] <= np.arange(L)[:, None]  # [L, S]

    ks = k[sampled]  # [S, B, D]
    vs = v[sampled]

    in_maps = []
    for b in range(B):
        mask_b = (causal_ok & (np.arange(S)[None, :] < mkl[b])).astype(f16)
        in_maps.append({
            "qT": np.ascontiguousarray(q[:, b, :].T.astype(f16)),
            "ksT": np.ascontiguousarray(ks[:, b, :].T.astype(f16)),
            "vsT": np.ascontiguousarray(vs[:, b, :].T.astype(f16)),
            "wq": wq_t, "wk": wk_t, "wv": wv_t, "wo": wo_t,
            "bq": bq8, "bk": bk_r, "bv": bv_r, "ob": ob_r,
            "mask": mask_b,
        })
    return in_maps


def kernel(q, k, v, key_length, sampled_index, in_proj_weight, in_proj_bias, out_w, out_b,
           **bench_kwargs):
    from concourse.bass_utils import run_bass_kernel_spmd

    sampled = np.asarray(sampled_index)
    key = ("nc", sampled.tobytes())
    if _CACHE.get("key") != key:
        _CACHE["nc"] = _build(_s_sizes(sampled))
        _CACHE["key"] = key
    nc = _CACHE["nc"]

    q = np.asarray(q, dtype=np.float32)
    k = np.asarray(k, dtype=np.float32)
    v = np.asarray(v, dtype=np.float32)
    in_maps = _prep_inputs(q, k, v, np.asarray(key_length), sampled,
                           np.asarray(in_proj_weight, dtype=np.float32),
                           np.asarray(in_proj_bias, dtype=np.float32),
                           np.asarray(out_w, dtype=np.float32),
                           np.asarray(out_b, dtype=np.float32))
    res = run_bass_kernel_spmd(nc, in_maps, list(range(B)), **bench_kwargs)
    _CACHE["last_results"] = res
    out = np.empty((L, B, D), dtype=np.float32)
    for b in range(B):
        out[:, b, :] = res.results[b]["out"]
    return out
